# revision 4
# baseline (speedup 1.0000x reference)
"""Trainium2 Bass kernel for the LIF spiking decoder (nn_Decoder_80736795230986).

Math per timestep t (T=16), batch-sharded over 8 cores (BS=512 rows each):
    c1 = x_t @ W1.T * 0.5            (charge, TAU=2 folded into weights)
    h1 = 0.5*v1 + c1                 (membrane)
    s1 = (h1 >= 0.5)                 (spike -> de_spk output)
    v1' = h1*(1-s1)                  (hard reset; state kept as u1 = 0.5*v1')
    h2/s2/v2 likewise from s1 @ Wout.T * 0.5
    votes += avgpool(s2)             (PSUM-resident accumulation over T)

Precision: matmul1 uses a 3-term float32r split (f32r rounds both operands
to 10-bit mantissa, exact MAC):  xh@Wh + xl@Wh + xh@Wl  with
xh = round10(x), xl = x - xh (exact).  Residual error ~1e-7 — fp32-grade.
matmul2 / vote run in bf16 (spikes are exactly 0/1; threshold margin 0.076).

Layouts are feature-major [feature, batch]: matmul1 output lands as
[h_tile(128 part), b(512 cols)] so the whole LIF update is elementwise on
natural tiles and s1 feeds matmul2 directly as the moving operand.
"""

import os
import sys
import types

import numpy as np

import concourse.bass as bass
import concourse.mybir as mybir
import concourse.tile as tile
from concourse import bacc
from concourse.bass_utils import run_bass_kernel_spmd

# ---------------------------------------------------------------- constants
B, T, D, H = 4096, 16, 512, 1024
NOUT, NVOTE = 110, 11
NCORES = 8
BS = B // NCORES            # 512 batch rows per core
KD = D // 128               # 4 contraction tiles for matmul1
NH = H // 128               # 8 h-tiles
NSTEPS = int(os.environ.get("KERNEL_NSTEPS", T))

TRACE = bool(int(os.environ.get("KERNEL_TRACE", "0")))
LAST_EXEC_NS = None

F32R = mybir.dt.float32r
F32 = mybir.dt.float32
BF16 = mybir.dt.bfloat16


def _install_ntff_hook():
    """Shim antenv.axon_hooks so trace=True can reach the axon NTFF profiler."""
    if "antenv.axon_hooks" in sys.modules:
        return
    m = types.ModuleType("antenv.axon_hooks")
    m._hook = None
    m.set_axon_ntff_profile_hook = lambda h: setattr(m, "_hook", h)
    m.get_axon_ntff_profile_hook = lambda: m._hook
    sys.modules["antenv.axon_hooks"] = m
    try:
        from trn_agent_boot.trn_boot import _ntff_profile_via_ctypes

        m.set_axon_ntff_profile_hook(
            _ntff_profile_via_ctypes("/opt/axon/libaxon_pjrt.so")
        )
    except Exception:
        pass


def _round10(a: np.ndarray) -> np.ndarray:
    """Round fp32 to 10 explicit mantissa bits (what f32r keeps)."""
    u = np.ascontiguousarray(a).view(np.uint32)
    out = ((u + np.uint32(1 << 12)) & np.uint32(0xFFFFE000)).view(np.float32)
    return out


def _build_program() -> bacc.Bacc:
    nc = bacc.Bacc("TRN2", target_bir_lowering=False, debug=False)

    xh_d = nc.dram_tensor("xh", [NSTEPS, 128, KD, BS], F32R, kind="ExternalInput")
    xl_d = nc.dram_tensor("xl", [NSTEPS, 128, KD, BS], F32R, kind="ExternalInput")
    wh_d = nc.dram_tensor("wh", [128, KD, H], F32R, kind="ExternalInput")
    wl_d = nc.dram_tensor("wl", [128, KD, H], F32R, kind="ExternalInput")
    wo_d = nc.dram_tensor("wo", [128, NH, NOUT], BF16, kind="ExternalInput")
    pool_d = nc.dram_tensor("pool", [NOUT, NVOTE], BF16, kind="ExternalInput")
    despk_d = nc.dram_tensor(
        "despk", [NSTEPS, 128, NH, BS], BF16, kind="ExternalOutput"
    )
    votes_d = nc.dram_tensor("votes", [NVOTE, BS], F32, kind="ExternalOutput")

    from contextlib import ExitStack

    with tile.TileContext(nc) as tc, ExitStack() as ctx:
        consts = ctx.enter_context(tc.tile_pool(name="consts", bufs=1))
        state = ctx.enter_context(tc.tile_pool(name="state", bufs=1))
        xpool = ctx.enter_context(tc.tile_pool(name="xpool", bufs=2))
        s1pool = ctx.enter_context(tc.tile_pool(name="s1pool", bufs=2))
        work = ctx.enter_context(tc.tile_pool(name="work", bufs=4))
        lif2 = ctx.enter_context(tc.tile_pool(name="lif2", bufs=2))
        pmm1 = ctx.enter_context(tc.tile_pool(name="pmm1", bufs=4, space="PSUM"))
        pmm2 = ctx.enter_context(tc.tile_pool(name="pmm2", bufs=2, space="PSUM"))
        pvote = ctx.enter_context(tc.tile_pool(name="pvote", bufs=1, space="PSUM"))

        wh = consts.tile([128, KD, H], F32R)
        wl = consts.tile([128, KD, H], F32R)
        wo = consts.tile([128, NH, NOUT], BF16)
        pmat = consts.tile([NOUT, NVOTE], BF16)
        nc.sync.dma_start(out=wh, in_=wh_d.ap())
        nc.sync.dma_start(out=wl, in_=wl_d.ap())
        nc.sync.dma_start(out=wo, in_=wo_d.ap())
        nc.sync.dma_start(out=pmat, in_=pool_d.ap())

        half = consts.tile([128, 1], F32)
        nc.vector.memset(half, 0.5)

        u1 = state.tile([128, NH, BS], F32)   # 0.5 * v1  per h-tile
        u2 = state.tile([NOUT, BS], F32)      # 0.5 * v2
        nc.vector.memset(u1, 0.0)
        nc.vector.memset(u2, 0.0)

        vote_ps = pvote.tile([NVOTE, BS], F32)

        for t in range(NSTEPS):
            xh = xpool.tile([128, KD, BS], F32R, tag="xh")
            xl = xpool.tile([128, KD, BS], F32R, tag="xl")
            nc.sync.dma_start(out=xh, in_=xh_d.ap()[t])
            nc.sync.dma_start(out=xl, in_=xl_d.ap()[t])

            s1 = s1pool.tile([128, NH, BS], BF16, tag="s1")

            for j in range(NH):
                ps = pmm1.tile([128, BS], F32, tag="ps1")
                n = 0
                for wsb, xsb in ((wh, xh), (wh, xl), (wl, xh)):
                    for k in range(KD):
                        nc.tensor.matmul(
                            ps,
                            wsb[:, k, bass.ts(j, 128)],
                            xsb[:, k, :],
                            start=(n == 0),
                            stop=(n == 3 * KD - 1),
                        )
                        n += 1
                # H = psum + u1[j]   (evacuates PSUM)
                ht = work.tile([128, BS], F32, tag="ht")
                nc.vector.tensor_tensor(
                    out=ht, in0=ps, in1=u1[:, j, :], op=mybir.AluOpType.add
                )
                # s1 = (H >= 0.5) as bf16 0/1  (de_spk output + mm2 operand)
                nc.vector.tensor_scalar(
                    s1[:, j, :], ht, 0.5, None, mybir.AluOpType.is_ge
                )
                # z = 0.5*(1-s1)  — ACT engine, Identity(-0.5*s1 + 0.5)
                zt = work.tile([128, BS], F32, tag="zt")
                nc.scalar.activation(
                    out=zt,
                    in_=s1[:, j, :],
                    func=mybir.ActivationFunctionType.Identity,
                    bias=half,
                    scale=-0.5,
                )
                # v1' = min(H, z):  H<0.5 -> H (no spike, z=0.5>H); else 0
                mt = work.tile([128, BS], F32, tag="mt")
                nc.vector.tensor_tensor(
                    out=mt, in0=ht, in1=zt, op=mybir.AluOpType.min
                )
                # u1[j] = 0.5 * v1'   — ACT engine
                nc.scalar.activation(
                    out=u1[:, j, :],
                    in_=mt,
                    func=mybir.ActivationFunctionType.Identity,
                    bias=0.0,
                    scale=0.5,
                )

            nc.sync.dma_start(out=despk_d.ap()[t], in_=s1)

            # ---- LIF 2 ----
            ps2 = pmm2.tile([NOUT, BS], F32, tag="ps2")
            for j in range(NH):
                nc.tensor.matmul(
                    ps2,
                    wo[:, j, :],
                    s1[:, j, :],
                    start=(j == 0),
                    stop=(j == NH - 1),
                )
            h2 = lif2.tile([NOUT, BS], F32, tag="h2")
            nc.vector.tensor_tensor(
                out=h2, in0=ps2, in1=u2, op=mybir.AluOpType.add
            )
            s2 = lif2.tile([NOUT, BS], BF16, tag="s2")
            nc.vector.tensor_scalar(s2, h2, 0.5, None, mybir.AluOpType.is_ge)
            z2 = lif2.tile([NOUT, BS], F32, tag="z2")
            nc.scalar.activation(
                out=z2,
                in_=s2,
                func=mybir.ActivationFunctionType.Identity,
                bias=half[:NOUT],
                scale=-0.5,
            )
            m2 = lif2.tile([NOUT, BS], F32, tag="m2")
            nc.vector.tensor_tensor(out=m2, in0=h2, in1=z2, op=mybir.AluOpType.min)
            nc.scalar.activation(
                out=u2,
                in_=m2,
                func=mybir.ActivationFunctionType.Identity,
                bias=0.0,
                scale=0.5,
            )
            # votes += pool.T @ s2   (accumulates in PSUM across all steps)
            nc.tensor.matmul(
                vote_ps,
                pmat,
                s2,
                start=(t == 0),
                stop=(t == NSTEPS - 1),
                skip_group_check=True,
            )

        vst = work.tile([NVOTE, BS], F32, tag="vst")
        nc.vector.tensor_copy(out=vst, in_=vote_ps)
        nc.sync.dma_start(out=votes_d.ap(), in_=vst)

    nc.compile()
    return nc


_PROGRAM = None


def kernel(x: np.ndarray, W1: np.ndarray, Wout: np.ndarray):
    global _PROGRAM, LAST_EXEC_NS
    import ml_dtypes

    x = np.asarray(x, dtype=np.float32)
    W1 = np.asarray(W1, dtype=np.float32)
    Wout = np.asarray(Wout, dtype=np.float32)

    # ---- host-side prep (weights, splits, layouts) ----
    w1t = np.ascontiguousarray(W1.T) * np.float32(0.5)          # [D, H], exact *0.5
    wh_full = _round10(w1t)
    wl_full = (w1t - wh_full).astype(np.float32)
    # [D, H] -> [128, KD, H]
    wh_a = np.ascontiguousarray(wh_full.reshape(KD, 128, H).transpose(1, 0, 2))
    wl_a = np.ascontiguousarray(wl_full.reshape(KD, 128, H).transpose(1, 0, 2))
    # WoutT' chunks: [H, NOUT] -> [128, NH, NOUT] bf16, *0.5 folded
    wot = (np.ascontiguousarray(Wout.T) * np.float32(0.5)).reshape(NH, 128, NOUT)
    wo_a = np.ascontiguousarray(wot.transpose(1, 0, 2)).astype(ml_dtypes.bfloat16)
    # pooling matrix: votes[c] = sum_t sum_j s2[10c+j] / (10*T)
    pool_a = np.zeros((NOUT, NVOTE), dtype=np.float32)
    for c in range(NVOTE):
        pool_a[c * 10 : (c + 1) * 10, c] = 1.0 / (10.0 * T)
    pool_a = pool_a.astype(ml_dtypes.bfloat16)

    # ---- per-core x shards: [BS, T, D] -> [T, 128, KD, BS] hi/lo ----
    in_maps = []
    for c in range(NCORES):
        xs = x[c * BS : (c + 1) * BS]                       # [BS, T, D]
        xt = np.ascontiguousarray(xs.transpose(1, 2, 0))    # [T, D, BS]
        xt = xt.reshape(T, KD, 128, BS).transpose(0, 2, 1, 3)  # [T,128,KD,BS]
        xt = np.ascontiguousarray(xt)[:NSTEPS]
        xh_a = _round10(xt)
        xl_a = (xt - xh_a).astype(np.float32)
        in_maps.append(
            {
                "xh": xh_a,
                "xl": xl_a,
                "wh": wh_a,
                "wl": wl_a,
                "wo": wo_a,
                "pool": pool_a,
            }
        )

    if _PROGRAM is None:
        _PROGRAM = _build_program()

    if TRACE:
        _install_ntff_hook()
    res = run_bass_kernel_spmd(
        _PROGRAM, in_maps, list(range(NCORES)), trace=TRACE
    )
    LAST_EXEC_NS = res.exec_time_ns

    # ---- gather / unshard ----
    out_spikes = np.empty((B, NVOTE), dtype=np.float32)
    de_spk = np.zeros((B, T, H), dtype=np.float32)
    for c in range(NCORES):
        r = res.results[c]
        out_spikes[c * BS : (c + 1) * BS] = r["votes"].T
        sp = np.asarray(r["despk"])                         # [NSTEPS,128,NH,BS] bf16
        de_spk[c * BS : (c + 1) * BS, :NSTEPS] = (
            sp.transpose(3, 0, 2, 1).reshape(BS, NSTEPS, H).astype(np.float32)
        )
    return out_spikes, de_spk


# revision 6
# speedup vs baseline: 1.0891x; 1.0891x over previous
"""Trainium2 Bass kernel for the LIF spiking decoder (nn_Decoder_80736795230986).

Math per timestep t (T=16), batch-sharded over 8 cores (BS=512 rows each):
    c1 = x_t @ W1.T * 0.5            (charge, TAU=2 folded into weights)
    h1 = 0.5*v1 + c1                 (membrane)
    s1 = (h1 >= 0.5)                 (spike -> de_spk output)
    v1' = h1*(1-s1)                  (hard reset; state kept as u1 = 0.5*v1')
    h2/s2/v2 likewise from s1 @ Wout.T * 0.5
    votes += avgpool(s2)             (PSUM-resident accumulation over T)

Precision: matmul1 uses a 3-term fp16 split (PE honors fp16 denormals,
fp32 MAC):  xh@Wh + xl@Wh + xh@Wl  with xh = fp16(x), xl = fp16(x - xh),
Wh = fp16(W'), Wl = fp16(W' - Wh).  Residual error ~3e-7 — fp32-grade.
matmul2 / vote run in fp16 (spikes are exactly 0/1; threshold margin 0.076).

Layouts are feature-major [feature, batch]: matmul1 output lands as
[h_tile(128 part), b(512 cols)] so the whole LIF update is elementwise on
natural tiles and s1 feeds matmul2 directly as the moving operand.
"""

import os
import sys
import types

import numpy as np

import concourse.bass as bass
import concourse.mybir as mybir
import concourse.tile as tile
from concourse import bacc
from concourse.bass_utils import run_bass_kernel_spmd

# ---------------------------------------------------------------- constants
B, T, D, H = 4096, 16, 512, 1024
NOUT, NVOTE = 110, 11
NCORES = 8
BS = B // NCORES            # 512 batch rows per core
KD = D // 128               # 4 contraction tiles for matmul1
NH = H // 128               # 8 h-tiles
NSTEPS = int(os.environ.get("KERNEL_NSTEPS", T))

TRACE = bool(int(os.environ.get("KERNEL_TRACE", "0")))
LAST_EXEC_NS = None
LAST_RESULTS = None

F32 = mybir.dt.float32
F16 = mybir.dt.float16
BF16 = mybir.dt.bfloat16


def _install_ntff_hook():
    """Shim antenv.axon_hooks so trace=True can reach the axon NTFF profiler."""
    if "antenv.axon_hooks" in sys.modules:
        return
    m = types.ModuleType("antenv.axon_hooks")
    m._hook = None
    m.set_axon_ntff_profile_hook = lambda h: setattr(m, "_hook", h)
    m.get_axon_ntff_profile_hook = lambda: m._hook
    sys.modules["antenv.axon_hooks"] = m
    try:
        from trn_agent_boot.trn_boot import _ntff_profile_via_ctypes

        m.set_axon_ntff_profile_hook(
            _ntff_profile_via_ctypes("/opt/axon/libaxon_pjrt.so")
        )
    except Exception:
        pass


def _round10(a: np.ndarray) -> np.ndarray:
    """Round fp32 to 10 explicit mantissa bits (what f32r keeps)."""
    u = np.ascontiguousarray(a).view(np.uint32)
    out = ((u + np.uint32(1 << 12)) & np.uint32(0xFFFFE000)).view(np.float32)
    return out


def _build_program() -> bacc.Bacc:
    nc = bacc.Bacc("TRN2", target_bir_lowering=False, debug=False)

    xh_d = nc.dram_tensor("xh", [NSTEPS, 128, KD, BS], F16, kind="ExternalInput")
    xl_d = nc.dram_tensor("xl", [NSTEPS, 128, KD, BS], F16, kind="ExternalInput")
    wh_d = nc.dram_tensor("wh", [128, KD, H], F16, kind="ExternalInput")
    wl_d = nc.dram_tensor("wl", [128, KD, H], F16, kind="ExternalInput")
    wo_d = nc.dram_tensor("wo", [128, NH, NOUT], F16, kind="ExternalInput")
    pool_d = nc.dram_tensor("pool", [NOUT, NVOTE], F16, kind="ExternalInput")
    despk_d = nc.dram_tensor(
        "despk", [NSTEPS, 128, NH, BS], F16, kind="ExternalOutput"
    )
    votes_d = nc.dram_tensor("votes", [NVOTE, BS], F32, kind="ExternalOutput")

    from contextlib import ExitStack

    with tile.TileContext(nc) as tc, ExitStack() as ctx:
        consts = ctx.enter_context(tc.tile_pool(name="consts", bufs=1))
        state = ctx.enter_context(tc.tile_pool(name="state", bufs=1))
        xpool = ctx.enter_context(tc.tile_pool(name="xpool", bufs=2))
        s1pool = ctx.enter_context(tc.tile_pool(name="s1pool", bufs=2))
        work = ctx.enter_context(tc.tile_pool(name="work", bufs=4))
        lif2 = ctx.enter_context(tc.tile_pool(name="lif2", bufs=2))
        pmm1 = ctx.enter_context(tc.tile_pool(name="pmm1", bufs=5, space="PSUM"))
        pmm2 = ctx.enter_context(tc.tile_pool(name="pmm2", bufs=2, space="PSUM"))
        pvote = ctx.enter_context(tc.tile_pool(name="pvote", bufs=1, space="PSUM"))

        wh = consts.tile([128, KD, H], F16)
        wl = consts.tile([128, KD, H], F16)
        wo = consts.tile([128, NH, NOUT], F16)
        pmat = consts.tile([NOUT, NVOTE], F16)
        nc.sync.dma_start(out=wh, in_=wh_d.ap())
        nc.sync.dma_start(out=wl, in_=wl_d.ap())
        nc.sync.dma_start(out=wo, in_=wo_d.ap())
        nc.sync.dma_start(out=pmat, in_=pool_d.ap())

        half = consts.tile([128, 1], F32)
        nc.vector.memset(half, 0.5)

        u1 = state.tile([128, NH, BS], F32)   # 0.5 * v1  per h-tile
        u2 = state.tile([NOUT, BS], F32)      # 0.5 * v2
        nc.vector.memset(u1, 0.0)
        nc.vector.memset(u2, 0.0)

        vote_ps = pvote.tile([NVOTE, BS], F32)

        for t in range(NSTEPS):
            xh = xpool.tile([128, KD, BS], F16, tag="xh")
            xl = xpool.tile([128, KD, BS], F16, tag="xl")
            nc.sync.dma_start(out=xh, in_=xh_d.ap()[t])
            nc.sync.dma_start(out=xl, in_=xl_d.ap()[t])

            s1 = s1pool.tile([128, NH, BS], F16, tag="s1")

            for j in range(NH):
                ps = pmm1.tile([128, BS], F32, tag="ps1")
                n = 0
                for wsb, xsb in ((wh, xh), (wh, xl), (wl, xh)):
                    for k in range(KD):
                        nc.tensor.matmul(
                            ps,
                            wsb[:, k, bass.ts(j, 128)],
                            xsb[:, k, :],
                            start=(n == 0),
                            stop=(n == 3 * KD - 1),
                        )
                        n += 1
                # H = psum + u1[j]   (evacuates PSUM)
                ht = work.tile([128, BS], F32, tag="ht")
                nc.vector.tensor_tensor(
                    out=ht, in0=ps, in1=u1[:, j, :], op=mybir.AluOpType.add
                )
                # s1 = (H >= 0.5) as bf16 0/1  (de_spk output + mm2 operand)
                nc.vector.tensor_scalar(
                    s1[:, j, :], ht, 0.5, None, mybir.AluOpType.is_ge
                )
                # z = 0.5*(1-s1)  — ACT engine, Identity(-0.5*s1 + 0.5)
                zt = work.tile([128, BS], F32, tag="zt")
                nc.scalar.activation(
                    out=zt,
                    in_=s1[:, j, :],
                    func=mybir.ActivationFunctionType.Identity,
                    bias=half,
                    scale=-0.5,
                )
                # v1' = min(H, z):  H<0.5 -> H (no spike, z=0.5>H); else 0
                mt = work.tile([128, BS], F32, tag="mt")
                nc.vector.tensor_tensor(
                    out=mt, in0=ht, in1=zt, op=mybir.AluOpType.min
                )
                # u1[j] = 0.5 * v1'   — ACT engine
                nc.scalar.activation(
                    out=u1[:, j, :],
                    in_=mt,
                    func=mybir.ActivationFunctionType.Identity,
                    bias=0.0,
                    scale=0.5,
                )

            nc.sync.dma_start(out=despk_d.ap()[t], in_=s1)

            # ---- LIF 2 ----
            ps2 = pmm2.tile([NOUT, BS], F32, tag="ps2")
            for j in range(NH):
                nc.tensor.matmul(
                    ps2,
                    wo[:, j, :],
                    s1[:, j, :],
                    start=(j == 0),
                    stop=(j == NH - 1),
                )
            h2 = lif2.tile([NOUT, BS], F32, tag="h2")
            nc.vector.tensor_tensor(
                out=h2, in0=ps2, in1=u2, op=mybir.AluOpType.add
            )
            s2 = lif2.tile([NOUT, BS], F16, tag="s2")
            nc.vector.tensor_scalar(s2, h2, 0.5, None, mybir.AluOpType.is_ge)
            z2 = lif2.tile([NOUT, BS], F32, tag="z2")
            nc.scalar.activation(
                out=z2,
                in_=s2,
                func=mybir.ActivationFunctionType.Identity,
                bias=half[:NOUT],
                scale=-0.5,
            )
            m2 = lif2.tile([NOUT, BS], F32, tag="m2")
            nc.vector.tensor_tensor(out=m2, in0=h2, in1=z2, op=mybir.AluOpType.min)
            nc.scalar.activation(
                out=u2,
                in_=m2,
                func=mybir.ActivationFunctionType.Identity,
                bias=0.0,
                scale=0.5,
            )
            # votes += pool.T @ s2   (accumulates in PSUM across all steps)
            nc.tensor.matmul(
                vote_ps,
                pmat,
                s2,
                start=(t == 0),
                stop=(t == NSTEPS - 1),
                skip_group_check=True,
            )

        vst = work.tile([NVOTE, BS], F32, tag="vst")
        nc.vector.tensor_copy(out=vst, in_=vote_ps)
        nc.sync.dma_start(out=votes_d.ap(), in_=vst)

    nc.compile()
    return nc


_PROGRAM = None


def kernel(x: np.ndarray, W1: np.ndarray, Wout: np.ndarray):
    global _PROGRAM, LAST_EXEC_NS
    import ml_dtypes

    x = np.asarray(x, dtype=np.float32)
    W1 = np.asarray(W1, dtype=np.float32)
    Wout = np.asarray(Wout, dtype=np.float32)

    # ---- host-side prep (weights, splits, layouts) ----
    w1t = np.ascontiguousarray(W1.T) * np.float32(0.5)          # [D, H], exact *0.5
    wh_full = w1t.astype(np.float16)
    wl_full = (w1t - wh_full.astype(np.float32)).astype(np.float16)
    # [D, H] -> [128, KD, H]
    wh_a = np.ascontiguousarray(wh_full.reshape(KD, 128, H).transpose(1, 0, 2))
    wl_a = np.ascontiguousarray(wl_full.reshape(KD, 128, H).transpose(1, 0, 2))
    # WoutT' chunks: [H, NOUT] -> [128, NH, NOUT] fp16, *0.5 folded
    wot = (np.ascontiguousarray(Wout.T) * np.float32(0.5)).reshape(NH, 128, NOUT)
    wo_a = np.ascontiguousarray(wot.transpose(1, 0, 2)).astype(np.float16)
    # pooling matrix: votes[c] = sum_t sum_j s2[10c+j] / (10*T)
    pool_a = np.zeros((NOUT, NVOTE), dtype=np.float32)
    for c in range(NVOTE):
        pool_a[c * 10 : (c + 1) * 10, c] = 1.0 / (10.0 * T)
    pool_a = pool_a.astype(np.float16)

    # ---- per-core x shards: [BS, T, D] -> [T, 128, KD, BS] hi/lo ----
    in_maps = []
    for c in range(NCORES):
        xs = x[c * BS : (c + 1) * BS]                       # [BS, T, D]
        xt = np.ascontiguousarray(xs.transpose(1, 2, 0))    # [T, D, BS]
        xt = xt.reshape(T, KD, 128, BS).transpose(0, 2, 1, 3)  # [T,128,KD,BS]
        xt = np.ascontiguousarray(xt)[:NSTEPS]
        xh_a = xt.astype(np.float16)
        xl_a = (xt - xh_a.astype(np.float32)).astype(np.float16)
        in_maps.append(
            {
                "xh": xh_a,
                "xl": xl_a,
                "wh": wh_a,
                "wl": wl_a,
                "wo": wo_a,
                "pool": pool_a,
            }
        )

    if _PROGRAM is None:
        _PROGRAM = _build_program()

    if TRACE:
        _install_ntff_hook()
    res = run_bass_kernel_spmd(
        _PROGRAM, in_maps, list(range(NCORES)), trace=TRACE
    )
    LAST_EXEC_NS = res.exec_time_ns
    globals()["LAST_RESULTS"] = res

    # ---- gather / unshard ----
    out_spikes = np.empty((B, NVOTE), dtype=np.float32)
    de_spk = np.zeros((B, T, H), dtype=np.float32)
    for c in range(NCORES):
        r = res.results[c]
        out_spikes[c * BS : (c + 1) * BS] = r["votes"].T
        sp = np.asarray(r["despk"])                         # [NSTEPS,128,NH,BS] bf16
        de_spk[c * BS : (c + 1) * BS, :NSTEPS] = (
            sp.transpose(3, 0, 2, 1).reshape(BS, NSTEPS, H).astype(np.float32)
        )
    return out_spikes, de_spk


# revision 7
# speedup vs baseline: 1.0959x; 1.0062x over previous
"""Trainium2 Bass kernel for the LIF spiking decoder (nn_Decoder_80736795230986).

Math per timestep t (T=16), batch-sharded over 8 cores (BS=512 rows each):
    c1 = x_t @ W1.T * 0.5            (charge, TAU=2 folded into weights)
    h1 = 0.5*v1 + c1                 (membrane)
    s1 = (h1 >= 0.5)                 (spike -> de_spk output)
    v1' = h1*(1-s1)                  (hard reset; state kept as u1 = 0.5*v1')
    h2/s2/v2 likewise from s1 @ Wout.T * 0.5
    votes += avgpool(s2)             (PSUM-resident accumulation over T)

Precision: matmul1 uses a 3-term fp16 split (PE honors fp16 denormals,
fp32 MAC):  xh@Wh + xl@Wh + xh@Wl  with xh = fp16(x), xl = fp16(x - xh),
Wh = fp16(W'), Wl = fp16(W' - Wh).  Residual error ~3e-7 — fp32-grade.
matmul2 / vote run in fp16 (spikes are exactly 0/1; threshold margin 0.076).

Layouts are feature-major [feature, batch]: matmul1 output lands as
[h_tile(128 part), b(512 cols)] so the whole LIF update is elementwise on
natural tiles and s1 feeds matmul2 directly as the moving operand.
"""

import os
import sys
import types

import numpy as np

import concourse.bass as bass
import concourse.mybir as mybir
import concourse.tile as tile
from concourse import bacc
from concourse.bass_utils import run_bass_kernel_spmd

# ---------------------------------------------------------------- constants
B, T, D, H = 4096, 16, 512, 1024
NOUT, NVOTE = 110, 11
NCORES = 8
BS = B // NCORES            # 512 batch rows per core
KD = D // 128               # 4 contraction tiles for matmul1
NH = H // 128               # 8 h-tiles
NSTEPS = int(os.environ.get("KERNEL_NSTEPS", T))

TRACE = bool(int(os.environ.get("KERNEL_TRACE", "0")))
LAST_EXEC_NS = None
LAST_RESULTS = None

F32 = mybir.dt.float32
F16 = mybir.dt.float16
BF16 = mybir.dt.bfloat16


def _install_ntff_hook():
    """Shim antenv.axon_hooks so trace=True can reach the axon NTFF profiler."""
    if "antenv.axon_hooks" in sys.modules:
        return
    m = types.ModuleType("antenv.axon_hooks")
    m._hook = None
    m.set_axon_ntff_profile_hook = lambda h: setattr(m, "_hook", h)
    m.get_axon_ntff_profile_hook = lambda: m._hook
    sys.modules["antenv.axon_hooks"] = m
    try:
        from trn_agent_boot.trn_boot import _ntff_profile_via_ctypes

        m.set_axon_ntff_profile_hook(
            _ntff_profile_via_ctypes("/opt/axon/libaxon_pjrt.so")
        )
    except Exception:
        pass


def _round10(a: np.ndarray) -> np.ndarray:
    """Round fp32 to 10 explicit mantissa bits (what f32r keeps)."""
    u = np.ascontiguousarray(a).view(np.uint32)
    out = ((u + np.uint32(1 << 12)) & np.uint32(0xFFFFE000)).view(np.float32)
    return out


def _build_program() -> bacc.Bacc:
    nc = bacc.Bacc("TRN2", target_bir_lowering=False, debug=False)

    xh_d = nc.dram_tensor("xh", [NSTEPS, 128, KD, BS], F16, kind="ExternalInput")
    xl_d = nc.dram_tensor("xl", [NSTEPS, 128, KD, BS], F16, kind="ExternalInput")
    wh_d = nc.dram_tensor("wh", [128, KD, H], F16, kind="ExternalInput")
    wl_d = nc.dram_tensor("wl", [128, KD, H], F16, kind="ExternalInput")
    wo_d = nc.dram_tensor("wo", [128, NH, NOUT], F16, kind="ExternalInput")
    pool_d = nc.dram_tensor("pool", [NOUT, NVOTE], F16, kind="ExternalInput")
    despk_d = nc.dram_tensor(
        "despk", [NSTEPS, 128, NH, BS], F16, kind="ExternalOutput"
    )
    votes_d = nc.dram_tensor("votes", [NVOTE, BS], F32, kind="ExternalOutput")

    from contextlib import ExitStack

    with tile.TileContext(nc) as tc, ExitStack() as ctx:
        consts = ctx.enter_context(tc.tile_pool(name="consts", bufs=1))
        state = ctx.enter_context(tc.tile_pool(name="state", bufs=1))
        xpool = ctx.enter_context(tc.tile_pool(name="xpool", bufs=3))
        s1pool = ctx.enter_context(tc.tile_pool(name="s1pool", bufs=3))
        work = ctx.enter_context(tc.tile_pool(name="work", bufs=4))
        lif2 = ctx.enter_context(tc.tile_pool(name="lif2", bufs=2))
        pmm1 = ctx.enter_context(tc.tile_pool(name="pmm1", bufs=5, space="PSUM"))
        pmm2 = ctx.enter_context(tc.tile_pool(name="pmm2", bufs=2, space="PSUM"))
        pvote = ctx.enter_context(tc.tile_pool(name="pvote", bufs=1, space="PSUM"))

        wh = consts.tile([128, KD, H], F16)
        wl = consts.tile([128, KD, H], F16)
        wo = consts.tile([128, NH, NOUT], F16)
        pmat = consts.tile([NOUT, NVOTE], F16)
        nc.sync.dma_start(out=wh, in_=wh_d.ap())
        nc.sync.dma_start(out=wl, in_=wl_d.ap())
        nc.sync.dma_start(out=wo, in_=wo_d.ap())
        nc.sync.dma_start(out=pmat, in_=pool_d.ap())

        half = consts.tile([128, 1], F32)
        nc.vector.memset(half, 0.5)

        u1 = state.tile([128, NH, BS], F32)   # 0.5 * v1  per h-tile
        u2 = state.tile([NOUT, BS], F32)      # 0.5 * v2
        nc.vector.memset(u1, 0.0)
        nc.vector.memset(u2, 0.0)

        vote_ps = pvote.tile([NVOTE, BS], F32)

        for t in range(NSTEPS):
            xh = xpool.tile([128, KD, BS], F16, tag="xh")
            xl = xpool.tile([128, KD, BS], F16, tag="xl")
            nc.sync.dma_start(out=xh, in_=xh_d.ap()[t])
            nc.sync.dma_start(out=xl, in_=xl_d.ap()[t])

            s1 = s1pool.tile([128, NH, BS], F16, tag="s1")

            for j in range(NH):
                ps = pmm1.tile([128, BS], F32, tag="ps1")
                n = 0
                for wsb, xsb in ((wh, xh), (wh, xl), (wl, xh)):
                    for k in range(KD):
                        nc.tensor.matmul(
                            ps,
                            wsb[:, k, bass.ts(j, 128)],
                            xsb[:, k, :],
                            start=(n == 0),
                            stop=(n == 3 * KD - 1),
                        )
                        n += 1
                # H = psum + u1[j]   (evacuates PSUM)
                ht = work.tile([128, BS], F32, tag="ht")
                nc.vector.tensor_tensor(
                    out=ht, in0=ps, in1=u1[:, j, :], op=mybir.AluOpType.add
                )
                # s1 = (H >= 0.5) as bf16 0/1  (de_spk output + mm2 operand)
                nc.vector.tensor_scalar(
                    s1[:, j, :], ht, 0.5, None, mybir.AluOpType.is_ge
                )
                # z = 0.5*(1-s1)  — ACT engine, Identity(-0.5*s1 + 0.5)
                zt = work.tile([128, BS], F32, tag="zt")
                nc.scalar.activation(
                    out=zt,
                    in_=s1[:, j, :],
                    func=mybir.ActivationFunctionType.Identity,
                    bias=half,
                    scale=-0.5,
                )
                # v1' = min(H, z):  H<0.5 -> H (no spike, z=0.5>H); else 0
                mt = work.tile([128, BS], F32, tag="mt")
                nc.vector.tensor_tensor(
                    out=mt, in0=ht, in1=zt, op=mybir.AluOpType.min
                )
                # u1[j] = 0.5 * v1'   — ACT engine
                nc.scalar.activation(
                    out=u1[:, j, :],
                    in_=mt,
                    func=mybir.ActivationFunctionType.Identity,
                    bias=0.0,
                    scale=0.5,
                )

            nc.sync.dma_start(out=despk_d.ap()[t], in_=s1)

            # ---- LIF 2 ----
            ps2 = pmm2.tile([NOUT, BS], F32, tag="ps2")
            for j in range(NH):
                nc.tensor.matmul(
                    ps2,
                    wo[:, j, :],
                    s1[:, j, :],
                    start=(j == 0),
                    stop=(j == NH - 1),
                )
            h2 = lif2.tile([NOUT, BS], F32, tag="h2")
            nc.vector.tensor_tensor(
                out=h2, in0=ps2, in1=u2, op=mybir.AluOpType.add
            )
            s2 = lif2.tile([NOUT, BS], F16, tag="s2")
            nc.vector.tensor_scalar(s2, h2, 0.5, None, mybir.AluOpType.is_ge)
            z2 = lif2.tile([NOUT, BS], F32, tag="z2")
            nc.scalar.activation(
                out=z2,
                in_=s2,
                func=mybir.ActivationFunctionType.Identity,
                bias=half[:NOUT],
                scale=-0.5,
            )
            m2 = lif2.tile([NOUT, BS], F32, tag="m2")
            nc.vector.tensor_tensor(out=m2, in0=h2, in1=z2, op=mybir.AluOpType.min)
            nc.scalar.activation(
                out=u2,
                in_=m2,
                func=mybir.ActivationFunctionType.Identity,
                bias=0.0,
                scale=0.5,
            )
            # votes += pool.T @ s2   (accumulates in PSUM across all steps)
            nc.tensor.matmul(
                vote_ps,
                pmat,
                s2,
                start=(t == 0),
                stop=(t == NSTEPS - 1),
                skip_group_check=True,
            )

        vst = work.tile([NVOTE, BS], F32, tag="vst")
        nc.vector.tensor_copy(out=vst, in_=vote_ps)
        nc.sync.dma_start(out=votes_d.ap(), in_=vst)

    nc.compile()
    return nc


_PROGRAM = None


def kernel(x: np.ndarray, W1: np.ndarray, Wout: np.ndarray):
    global _PROGRAM, LAST_EXEC_NS
    import ml_dtypes

    x = np.asarray(x, dtype=np.float32)
    W1 = np.asarray(W1, dtype=np.float32)
    Wout = np.asarray(Wout, dtype=np.float32)

    # ---- host-side prep (weights, splits, layouts) ----
    w1t = np.ascontiguousarray(W1.T) * np.float32(0.5)          # [D, H], exact *0.5
    wh_full = w1t.astype(np.float16)
    wl_full = (w1t - wh_full.astype(np.float32)).astype(np.float16)
    # [D, H] -> [128, KD, H]
    wh_a = np.ascontiguousarray(wh_full.reshape(KD, 128, H).transpose(1, 0, 2))
    wl_a = np.ascontiguousarray(wl_full.reshape(KD, 128, H).transpose(1, 0, 2))
    # WoutT' chunks: [H, NOUT] -> [128, NH, NOUT] fp16, *0.5 folded
    wot = (np.ascontiguousarray(Wout.T) * np.float32(0.5)).reshape(NH, 128, NOUT)
    wo_a = np.ascontiguousarray(wot.transpose(1, 0, 2)).astype(np.float16)
    # pooling matrix: votes[c] = sum_t sum_j s2[10c+j] / (10*T)
    pool_a = np.zeros((NOUT, NVOTE), dtype=np.float32)
    for c in range(NVOTE):
        pool_a[c * 10 : (c + 1) * 10, c] = 1.0 / (10.0 * T)
    pool_a = pool_a.astype(np.float16)

    # ---- per-core x shards: [BS, T, D] -> [T, 128, KD, BS] hi/lo ----
    in_maps = []
    for c in range(NCORES):
        xs = x[c * BS : (c + 1) * BS]                       # [BS, T, D]
        xt = np.ascontiguousarray(xs.transpose(1, 2, 0))    # [T, D, BS]
        xt = xt.reshape(T, KD, 128, BS).transpose(0, 2, 1, 3)  # [T,128,KD,BS]
        xt = np.ascontiguousarray(xt)[:NSTEPS]
        xh_a = xt.astype(np.float16)
        xl_a = (xt - xh_a.astype(np.float32)).astype(np.float16)
        in_maps.append(
            {
                "xh": xh_a,
                "xl": xl_a,
                "wh": wh_a,
                "wl": wl_a,
                "wo": wo_a,
                "pool": pool_a,
            }
        )

    if _PROGRAM is None:
        _PROGRAM = _build_program()

    if TRACE:
        _install_ntff_hook()
    res = run_bass_kernel_spmd(
        _PROGRAM, in_maps, list(range(NCORES)), trace=TRACE
    )
    LAST_EXEC_NS = res.exec_time_ns
    globals()["LAST_RESULTS"] = res

    # ---- gather / unshard ----
    out_spikes = np.empty((B, NVOTE), dtype=np.float32)
    de_spk = np.zeros((B, T, H), dtype=np.float32)
    for c in range(NCORES):
        r = res.results[c]
        out_spikes[c * BS : (c + 1) * BS] = r["votes"].T
        sp = np.asarray(r["despk"])                         # [NSTEPS,128,NH,BS] bf16
        de_spk[c * BS : (c + 1) * BS, :NSTEPS] = (
            sp.transpose(3, 0, 2, 1).reshape(BS, NSTEPS, H).astype(np.float32)
        )
    return out_spikes, de_spk


# revision 8
# speedup vs baseline: 1.0966x; 1.0006x over previous
"""Trainium2 Bass kernel for the LIF spiking decoder (nn_Decoder_80736795230986).

Math per timestep t (T=16), batch-sharded over 8 cores (BS=512 rows each):
    c1 = x_t @ W1.T * 0.5            (charge, TAU=2 folded into weights)
    h1 = 0.5*v1 + c1                 (membrane)
    s1 = (h1 >= 0.5)                 (spike -> de_spk output)
    v1' = h1*(1-s1)                  (hard reset; state kept as u1 = 0.5*v1')
    h2/s2/v2 likewise from s1 @ Wout.T * 0.5
    votes += avgpool(s2)             (PSUM-resident accumulation over T)

Precision: matmul1 uses a 3-term fp16 split (PE honors fp16 denormals,
fp32 MAC):  xh@Wh + xl@Wh + xh@Wl  with xh = fp16(x), xl = fp16(x - xh),
Wh = fp16(W'), Wl = fp16(W' - Wh).  Residual error ~3e-7 — fp32-grade.
matmul2 / vote run in fp16 (spikes are exactly 0/1; threshold margin 0.076).

Layouts are feature-major [feature, batch]: matmul1 output lands as
[h_tile(128 part), b(512 cols)] so the whole LIF update is elementwise on
natural tiles and s1 feeds matmul2 directly as the moving operand.
"""

import os
import sys
import types

import numpy as np

import concourse.bass as bass
import concourse.mybir as mybir
import concourse.tile as tile
from concourse import bacc
from concourse.bass_utils import run_bass_kernel_spmd

# ---------------------------------------------------------------- constants
B, T, D, H = 4096, 16, 512, 1024
NOUT, NVOTE = 110, 11
NCORES = 8
BS = B // NCORES            # 512 batch rows per core
KD = D // 128               # 4 contraction tiles for matmul1
NH = H // 128               # 8 h-tiles
NSTEPS = int(os.environ.get("KERNEL_NSTEPS", T))

TRACE = bool(int(os.environ.get("KERNEL_TRACE", "0")))
LAST_EXEC_NS = None
LAST_RESULTS = None

F32 = mybir.dt.float32
F16 = mybir.dt.float16
BF16 = mybir.dt.bfloat16


def _install_ntff_hook():
    """Shim antenv.axon_hooks so trace=True can reach the axon NTFF profiler."""
    if "antenv.axon_hooks" in sys.modules:
        return
    m = types.ModuleType("antenv.axon_hooks")
    m._hook = None
    m.set_axon_ntff_profile_hook = lambda h: setattr(m, "_hook", h)
    m.get_axon_ntff_profile_hook = lambda: m._hook
    sys.modules["antenv.axon_hooks"] = m
    try:
        from trn_agent_boot.trn_boot import _ntff_profile_via_ctypes

        m.set_axon_ntff_profile_hook(
            _ntff_profile_via_ctypes("/opt/axon/libaxon_pjrt.so")
        )
    except Exception:
        pass


def _round10(a: np.ndarray) -> np.ndarray:
    """Round fp32 to 10 explicit mantissa bits (what f32r keeps)."""
    u = np.ascontiguousarray(a).view(np.uint32)
    out = ((u + np.uint32(1 << 12)) & np.uint32(0xFFFFE000)).view(np.float32)
    return out


def _build_program() -> bacc.Bacc:
    nc = bacc.Bacc("TRN2", target_bir_lowering=False, debug=False)

    xh_d = nc.dram_tensor("xh", [NSTEPS, 128, KD, BS], F16, kind="ExternalInput")
    xl_d = nc.dram_tensor("xl", [NSTEPS, 128, KD, BS], F16, kind="ExternalInput")
    wh_d = nc.dram_tensor("wh", [128, KD, H], F16, kind="ExternalInput")
    wl_d = nc.dram_tensor("wl", [128, KD, H], F16, kind="ExternalInput")
    wo_d = nc.dram_tensor("wo", [128, NH, NOUT], F16, kind="ExternalInput")
    pool_d = nc.dram_tensor("pool", [NOUT, NVOTE], F16, kind="ExternalInput")
    despk_d = nc.dram_tensor(
        "despk", [NSTEPS, 128, NH, BS], F16, kind="ExternalOutput"
    )
    votes_d = nc.dram_tensor("votes", [NVOTE, BS], F32, kind="ExternalOutput")

    from contextlib import ExitStack

    with tile.TileContext(nc) as tc, ExitStack() as ctx:
        consts = ctx.enter_context(tc.tile_pool(name="consts", bufs=1))
        state = ctx.enter_context(tc.tile_pool(name="state", bufs=1))
        xpool = ctx.enter_context(tc.tile_pool(name="xpool", bufs=3))
        s1pool = ctx.enter_context(tc.tile_pool(name="s1pool", bufs=3))
        work = ctx.enter_context(tc.tile_pool(name="work", bufs=4))
        lif2 = ctx.enter_context(tc.tile_pool(name="lif2", bufs=2))
        pmm1 = ctx.enter_context(tc.tile_pool(name="pmm1", bufs=5, space="PSUM"))
        pmm2 = ctx.enter_context(tc.tile_pool(name="pmm2", bufs=2, space="PSUM"))
        pvote = ctx.enter_context(tc.tile_pool(name="pvote", bufs=1, space="PSUM"))

        wh = consts.tile([128, KD, H], F16)
        wl = consts.tile([128, KD, H], F16)
        wo = consts.tile([128, NH, NOUT], F16)
        pmat = consts.tile([NOUT, NVOTE], F16)
        nc.sync.dma_start(out=wh, in_=wh_d.ap())

        half = consts.tile([128, 1], F32)
        nc.vector.memset(half, 0.5)

        u1 = state.tile([128, NH, BS], F32)   # 0.5 * v1  per h-tile
        u2 = state.tile([NOUT, BS], F32)      # 0.5 * v2
        nc.vector.memset(u1, 0.0)
        nc.vector.memset(u2, 0.0)

        vote_ps = pvote.tile([NVOTE, BS], F32)

        for t in range(NSTEPS):
            xh = xpool.tile([128, KD, BS], F16, tag="xh")
            xl = xpool.tile([128, KD, BS], F16, tag="xl")
            nc.sync.dma_start(out=xh, in_=xh_d.ap()[t])
            nc.sync.dma_start(out=xl, in_=xl_d.ap()[t])
            if t == 0:
                # deferred const loads: first mm1 terms only need wh + x[0]
                nc.sync.dma_start(out=wl, in_=wl_d.ap())
                nc.sync.dma_start(out=wo, in_=wo_d.ap())
                nc.sync.dma_start(out=pmat, in_=pool_d.ap())

            s1 = s1pool.tile([128, NH, BS], F16, tag="s1")

            for j in range(NH):
                ps = pmm1.tile([128, BS], F32, tag="ps1")
                n = 0
                for wsb, xsb in ((wh, xh), (wh, xl), (wl, xh)):
                    for k in range(KD):
                        nc.tensor.matmul(
                            ps,
                            wsb[:, k, bass.ts(j, 128)],
                            xsb[:, k, :],
                            start=(n == 0),
                            stop=(n == 3 * KD - 1),
                        )
                        n += 1
                # H = psum + u1[j]   (evacuates PSUM)
                ht = work.tile([128, BS], F32, tag="ht")
                nc.vector.tensor_tensor(
                    out=ht, in0=ps, in1=u1[:, j, :], op=mybir.AluOpType.add
                )
                # s1 = (H >= 0.5) as bf16 0/1  (de_spk output + mm2 operand)
                nc.vector.tensor_scalar(
                    s1[:, j, :], ht, 0.5, None, mybir.AluOpType.is_ge
                )
                # z = 0.5*(1-s1)  — ACT engine, Identity(-0.5*s1 + 0.5)
                zt = work.tile([128, BS], F32, tag="zt")
                nc.scalar.activation(
                    out=zt,
                    in_=s1[:, j, :],
                    func=mybir.ActivationFunctionType.Identity,
                    bias=half,
                    scale=-0.5,
                )
                # v1' = min(H, z):  H<0.5 -> H (no spike, z=0.5>H); else 0
                mt = work.tile([128, BS], F32, tag="mt")
                nc.vector.tensor_tensor(
                    out=mt, in0=ht, in1=zt, op=mybir.AluOpType.min
                )
                # u1[j] = 0.5 * v1'   — ACT engine
                nc.scalar.activation(
                    out=u1[:, j, :],
                    in_=mt,
                    func=mybir.ActivationFunctionType.Identity,
                    bias=0.0,
                    scale=0.5,
                )

            nc.sync.dma_start(out=despk_d.ap()[t], in_=s1)

            # ---- LIF 2 ----
            ps2 = pmm2.tile([NOUT, BS], F32, tag="ps2")
            for j in range(NH):
                nc.tensor.matmul(
                    ps2,
                    wo[:, j, :],
                    s1[:, j, :],
                    start=(j == 0),
                    stop=(j == NH - 1),
                )
            h2 = lif2.tile([NOUT, BS], F32, tag="h2")
            nc.vector.tensor_tensor(
                out=h2, in0=ps2, in1=u2, op=mybir.AluOpType.add
            )
            s2 = lif2.tile([NOUT, BS], F16, tag="s2")
            nc.vector.tensor_scalar(s2, h2, 0.5, None, mybir.AluOpType.is_ge)
            z2 = lif2.tile([NOUT, BS], F32, tag="z2")
            nc.scalar.activation(
                out=z2,
                in_=s2,
                func=mybir.ActivationFunctionType.Identity,
                bias=half[:NOUT],
                scale=-0.5,
            )
            m2 = lif2.tile([NOUT, BS], F32, tag="m2")
            nc.vector.tensor_tensor(out=m2, in0=h2, in1=z2, op=mybir.AluOpType.min)
            nc.scalar.activation(
                out=u2,
                in_=m2,
                func=mybir.ActivationFunctionType.Identity,
                bias=0.0,
                scale=0.5,
            )
            # votes += pool.T @ s2   (accumulates in PSUM across all steps)
            nc.tensor.matmul(
                vote_ps,
                pmat,
                s2,
                start=(t == 0),
                stop=(t == NSTEPS - 1),
                skip_group_check=True,
            )

        vst = work.tile([NVOTE, BS], F32, tag="vst")
        nc.vector.tensor_copy(out=vst, in_=vote_ps)
        nc.sync.dma_start(out=votes_d.ap(), in_=vst)

    nc.compile()
    return nc


_PROGRAM = None


def kernel(x: np.ndarray, W1: np.ndarray, Wout: np.ndarray):
    global _PROGRAM, LAST_EXEC_NS
    import ml_dtypes

    x = np.asarray(x, dtype=np.float32)
    W1 = np.asarray(W1, dtype=np.float32)
    Wout = np.asarray(Wout, dtype=np.float32)

    # ---- host-side prep (weights, splits, layouts) ----
    w1t = np.ascontiguousarray(W1.T) * np.float32(0.5)          # [D, H], exact *0.5
    wh_full = w1t.astype(np.float16)
    wl_full = (w1t - wh_full.astype(np.float32)).astype(np.float16)
    # [D, H] -> [128, KD, H]
    wh_a = np.ascontiguousarray(wh_full.reshape(KD, 128, H).transpose(1, 0, 2))
    wl_a = np.ascontiguousarray(wl_full.reshape(KD, 128, H).transpose(1, 0, 2))
    # WoutT' chunks: [H, NOUT] -> [128, NH, NOUT] fp16, *0.5 folded
    wot = (np.ascontiguousarray(Wout.T) * np.float32(0.5)).reshape(NH, 128, NOUT)
    wo_a = np.ascontiguousarray(wot.transpose(1, 0, 2)).astype(np.float16)
    # pooling matrix: votes[c] = sum_t sum_j s2[10c+j] / (10*T)
    pool_a = np.zeros((NOUT, NVOTE), dtype=np.float32)
    for c in range(NVOTE):
        pool_a[c * 10 : (c + 1) * 10, c] = 1.0 / (10.0 * T)
    pool_a = pool_a.astype(np.float16)

    # ---- per-core x shards: [BS, T, D] -> [T, 128, KD, BS] hi/lo ----
    in_maps = []
    for c in range(NCORES):
        xs = x[c * BS : (c + 1) * BS]                       # [BS, T, D]
        xt = np.ascontiguousarray(xs.transpose(1, 2, 0))    # [T, D, BS]
        xt = xt.reshape(T, KD, 128, BS).transpose(0, 2, 1, 3)  # [T,128,KD,BS]
        xt = np.ascontiguousarray(xt)[:NSTEPS]
        xh_a = xt.astype(np.float16)
        xl_a = (xt - xh_a.astype(np.float32)).astype(np.float16)
        in_maps.append(
            {
                "xh": xh_a,
                "xl": xl_a,
                "wh": wh_a,
                "wl": wl_a,
                "wo": wo_a,
                "pool": pool_a,
            }
        )

    if _PROGRAM is None:
        _PROGRAM = _build_program()

    if TRACE:
        _install_ntff_hook()
    res = run_bass_kernel_spmd(
        _PROGRAM, in_maps, list(range(NCORES)), trace=TRACE
    )
    LAST_EXEC_NS = res.exec_time_ns
    globals()["LAST_RESULTS"] = res

    # ---- gather / unshard ----
    out_spikes = np.empty((B, NVOTE), dtype=np.float32)
    de_spk = np.zeros((B, T, H), dtype=np.float32)
    for c in range(NCORES):
        r = res.results[c]
        out_spikes[c * BS : (c + 1) * BS] = r["votes"].T
        sp = np.asarray(r["despk"])                         # [NSTEPS,128,NH,BS] bf16
        de_spk[c * BS : (c + 1) * BS, :NSTEPS] = (
            sp.transpose(3, 0, 2, 1).reshape(BS, NSTEPS, H).astype(np.float32)
        )
    return out_spikes, de_spk


# revision 9
# speedup vs baseline: 1.1049x; 1.0076x over previous
"""Trainium2 Bass kernel for the LIF spiking decoder (nn_Decoder_80736795230986).

Math per timestep t (T=16), batch-sharded over 8 cores (BS=512 rows each):
    c1 = x_t @ W1.T * 0.5            (charge, TAU=2 folded into weights)
    h1 = 0.5*v1 + c1                 (membrane)
    s1 = (h1 >= 0.5)                 (spike -> de_spk output)
    v1' = h1*(1-s1)                  (hard reset; state kept as u1 = 0.5*v1')
    h2/s2/v2 likewise from s1 @ Wout.T * 0.5
    votes += avgpool(s2)             (PSUM-resident accumulation over T)

Precision: matmul1 uses a 3-term fp16 split (PE honors fp16 denormals,
fp32 MAC):  xh@Wh + xl@Wh + xh@Wl  with xh = fp16(x), xl = fp16(x - xh),
Wh = fp16(W'), Wl = fp16(W' - Wh).  Residual error ~3e-7 — fp32-grade.
matmul2 / vote run in fp16 (spikes are exactly 0/1; threshold margin 0.076).

Layouts are feature-major [feature, batch]: matmul1 output lands as
[h_tile(128 part), b(512 cols)] so the whole LIF update is elementwise on
natural tiles and s1 feeds matmul2 directly as the moving operand.
"""

import os
import sys
import types

import numpy as np

import concourse.bass as bass
import concourse.mybir as mybir
import concourse.tile as tile
from concourse import bacc
from concourse.bass_utils import run_bass_kernel_spmd

# ---------------------------------------------------------------- constants
B, T, D, H = 4096, 16, 512, 1024
NOUT, NVOTE = 110, 11
NCORES = 8
BS = B // NCORES            # 512 batch rows per core
KD = D // 128               # 4 contraction tiles for matmul1
NH = H // 128               # 8 h-tiles
NSTEPS = int(os.environ.get("KERNEL_NSTEPS", T))

TRACE = bool(int(os.environ.get("KERNEL_TRACE", "0")))
LAST_EXEC_NS = None
LAST_RESULTS = None

F32 = mybir.dt.float32
F16 = mybir.dt.float16
BF16 = mybir.dt.bfloat16


def _install_ntff_hook():
    """Shim antenv.axon_hooks so trace=True can reach the axon NTFF profiler."""
    if "antenv.axon_hooks" in sys.modules:
        return
    m = types.ModuleType("antenv.axon_hooks")
    m._hook = None
    m.set_axon_ntff_profile_hook = lambda h: setattr(m, "_hook", h)
    m.get_axon_ntff_profile_hook = lambda: m._hook
    sys.modules["antenv.axon_hooks"] = m
    try:
        from trn_agent_boot.trn_boot import _ntff_profile_via_ctypes

        m.set_axon_ntff_profile_hook(
            _ntff_profile_via_ctypes("/opt/axon/libaxon_pjrt.so")
        )
    except Exception:
        pass


def _round10(a: np.ndarray) -> np.ndarray:
    """Round fp32 to 10 explicit mantissa bits (what f32r keeps)."""
    u = np.ascontiguousarray(a).view(np.uint32)
    out = ((u + np.uint32(1 << 12)) & np.uint32(0xFFFFE000)).view(np.float32)
    return out


def _build_program() -> bacc.Bacc:
    nc = bacc.Bacc("TRN2", target_bir_lowering=False, debug=False)

    xh_d = nc.dram_tensor("xh", [NSTEPS, 128, KD, BS], F16, kind="ExternalInput")
    xl_d = nc.dram_tensor("xl", [NSTEPS, 128, KD, BS], F16, kind="ExternalInput")
    wh_d = nc.dram_tensor("wh", [128, KD, H], F16, kind="ExternalInput")
    wl_d = nc.dram_tensor("wl", [128, KD, H], F16, kind="ExternalInput")
    wo_d = nc.dram_tensor("wo", [128, NH, NOUT], F16, kind="ExternalInput")
    pool_d = nc.dram_tensor("pool", [NOUT, NVOTE], F16, kind="ExternalInput")
    despk_d = nc.dram_tensor(
        "despk", [NSTEPS, 128, NH, BS], F16, kind="ExternalOutput"
    )
    votes_d = nc.dram_tensor("votes", [NVOTE, BS], F32, kind="ExternalOutput")

    from contextlib import ExitStack

    with tile.TileContext(nc) as tc, ExitStack() as ctx:
        consts = ctx.enter_context(tc.tile_pool(name="consts", bufs=1))
        state = ctx.enter_context(tc.tile_pool(name="state", bufs=1))
        xpool = ctx.enter_context(tc.tile_pool(name="xpool", bufs=3))
        s1pool = ctx.enter_context(tc.tile_pool(name="s1pool", bufs=3))
        work = ctx.enter_context(tc.tile_pool(name="work", bufs=4))
        lif2 = ctx.enter_context(tc.tile_pool(name="lif2", bufs=2))
        pmm1 = ctx.enter_context(tc.tile_pool(name="pmm1", bufs=6, space="PSUM"))
        pmm2 = ctx.enter_context(tc.tile_pool(name="pmm2", bufs=1, space="PSUM"))
        pvote = ctx.enter_context(tc.tile_pool(name="pvote", bufs=1, space="PSUM"))

        wh = consts.tile([128, KD, H], F16)
        wl = consts.tile([128, KD, H], F16)
        wo = consts.tile([128, NH, NOUT], F16)
        pmat = consts.tile([NOUT, NVOTE], F16)
        nc.sync.dma_start(out=wh, in_=wh_d.ap())

        half = consts.tile([128, 1], F32)
        nc.vector.memset(half, 0.5)

        u1 = state.tile([128, NH, BS], F32)   # 0.5 * v1  per h-tile
        u2 = state.tile([NOUT, BS], F32)      # 0.5 * v2
        nc.vector.memset(u1, 0.0)
        nc.vector.memset(u2, 0.0)

        vote_ps = pvote.tile([NVOTE, BS], F32)

        for t in range(NSTEPS):
            xh = xpool.tile([128, KD, BS], F16, tag="xh")
            xl = xpool.tile([128, KD, BS], F16, tag="xl")
            nc.sync.dma_start(out=xh, in_=xh_d.ap()[t])
            nc.sync.dma_start(out=xl, in_=xl_d.ap()[t])
            if t == 0:
                # deferred const loads: first mm1 terms only need wh + x[0]
                nc.sync.dma_start(out=wl, in_=wl_d.ap())
                nc.sync.dma_start(out=wo, in_=wo_d.ap())
                nc.sync.dma_start(out=pmat, in_=pool_d.ap())

            s1 = s1pool.tile([128, NH, BS], F16, tag="s1")

            for j in range(NH):
                ps = pmm1.tile([128, BS], F32, tag="ps1")
                n = 0
                for wsb, xsb in ((wh, xh), (wh, xl), (wl, xh)):
                    for k in range(KD):
                        nc.tensor.matmul(
                            ps,
                            wsb[:, k, bass.ts(j, 128)],
                            xsb[:, k, :],
                            start=(n == 0),
                            stop=(n == 3 * KD - 1),
                        )
                        n += 1
                # H = psum + u1[j]   (evacuates PSUM)
                ht = work.tile([128, BS], F32, tag="ht")
                nc.vector.tensor_tensor(
                    out=ht, in0=ps, in1=u1[:, j, :], op=mybir.AluOpType.add
                )
                # s1 = (H >= 0.5) as bf16 0/1  (de_spk output + mm2 operand)
                nc.vector.tensor_scalar(
                    s1[:, j, :], ht, 0.5, None, mybir.AluOpType.is_ge
                )
                # z = 0.5*(1-s1)  — ACT engine, Identity(-0.5*s1 + 0.5)
                zt = work.tile([128, BS], F32, tag="zt")
                nc.scalar.activation(
                    out=zt,
                    in_=s1[:, j, :],
                    func=mybir.ActivationFunctionType.Identity,
                    bias=half,
                    scale=-0.5,
                )
                # v1' = min(H, z):  H<0.5 -> H (no spike, z=0.5>H); else 0
                mt = work.tile([128, BS], F32, tag="mt")
                nc.vector.tensor_tensor(
                    out=mt, in0=ht, in1=zt, op=mybir.AluOpType.min
                )
                # u1[j] = 0.5 * v1'   — ACT engine
                nc.scalar.activation(
                    out=u1[:, j, :],
                    in_=mt,
                    func=mybir.ActivationFunctionType.Identity,
                    bias=0.0,
                    scale=0.5,
                )

            nc.sync.dma_start(out=despk_d.ap()[t], in_=s1)

            # ---- LIF 2 ----
            ps2 = pmm2.tile([NOUT, BS], F32, tag="ps2")
            for j in range(NH):
                nc.tensor.matmul(
                    ps2,
                    wo[:, j, :],
                    s1[:, j, :],
                    start=(j == 0),
                    stop=(j == NH - 1),
                )
            h2 = lif2.tile([NOUT, BS], F32, tag="h2")
            nc.vector.tensor_tensor(
                out=h2, in0=ps2, in1=u2, op=mybir.AluOpType.add
            )
            s2 = lif2.tile([NOUT, BS], F16, tag="s2")
            nc.vector.tensor_scalar(s2, h2, 0.5, None, mybir.AluOpType.is_ge)
            z2 = lif2.tile([NOUT, BS], F32, tag="z2")
            nc.scalar.activation(
                out=z2,
                in_=s2,
                func=mybir.ActivationFunctionType.Identity,
                bias=half[:NOUT],
                scale=-0.5,
            )
            m2 = lif2.tile([NOUT, BS], F32, tag="m2")
            nc.vector.tensor_tensor(out=m2, in0=h2, in1=z2, op=mybir.AluOpType.min)
            nc.scalar.activation(
                out=u2,
                in_=m2,
                func=mybir.ActivationFunctionType.Identity,
                bias=0.0,
                scale=0.5,
            )
            # votes += pool.T @ s2   (accumulates in PSUM across all steps)
            nc.tensor.matmul(
                vote_ps,
                pmat,
                s2,
                start=(t == 0),
                stop=(t == NSTEPS - 1),
                skip_group_check=True,
            )

        vst = work.tile([NVOTE, BS], F32, tag="vst")
        nc.vector.tensor_copy(out=vst, in_=vote_ps)
        nc.sync.dma_start(out=votes_d.ap(), in_=vst)

    nc.compile()
    return nc


_PROGRAM = None


def kernel(x: np.ndarray, W1: np.ndarray, Wout: np.ndarray):
    global _PROGRAM, LAST_EXEC_NS
    import ml_dtypes

    x = np.asarray(x, dtype=np.float32)
    W1 = np.asarray(W1, dtype=np.float32)
    Wout = np.asarray(Wout, dtype=np.float32)

    # ---- host-side prep (weights, splits, layouts) ----
    w1t = np.ascontiguousarray(W1.T) * np.float32(0.5)          # [D, H], exact *0.5
    wh_full = w1t.astype(np.float16)
    wl_full = (w1t - wh_full.astype(np.float32)).astype(np.float16)
    # [D, H] -> [128, KD, H]
    wh_a = np.ascontiguousarray(wh_full.reshape(KD, 128, H).transpose(1, 0, 2))
    wl_a = np.ascontiguousarray(wl_full.reshape(KD, 128, H).transpose(1, 0, 2))
    # WoutT' chunks: [H, NOUT] -> [128, NH, NOUT] fp16, *0.5 folded
    wot = (np.ascontiguousarray(Wout.T) * np.float32(0.5)).reshape(NH, 128, NOUT)
    wo_a = np.ascontiguousarray(wot.transpose(1, 0, 2)).astype(np.float16)
    # pooling matrix: votes[c] = sum_t sum_j s2[10c+j] / (10*T)
    pool_a = np.zeros((NOUT, NVOTE), dtype=np.float32)
    for c in range(NVOTE):
        pool_a[c * 10 : (c + 1) * 10, c] = 1.0 / (10.0 * T)
    pool_a = pool_a.astype(np.float16)

    # ---- per-core x shards: [BS, T, D] -> [T, 128, KD, BS] hi/lo ----
    in_maps = []
    for c in range(NCORES):
        xs = x[c * BS : (c + 1) * BS]                       # [BS, T, D]
        xt = np.ascontiguousarray(xs.transpose(1, 2, 0))    # [T, D, BS]
        xt = xt.reshape(T, KD, 128, BS).transpose(0, 2, 1, 3)  # [T,128,KD,BS]
        xt = np.ascontiguousarray(xt)[:NSTEPS]
        xh_a = xt.astype(np.float16)
        xl_a = (xt - xh_a.astype(np.float32)).astype(np.float16)
        in_maps.append(
            {
                "xh": xh_a,
                "xl": xl_a,
                "wh": wh_a,
                "wl": wl_a,
                "wo": wo_a,
                "pool": pool_a,
            }
        )

    if _PROGRAM is None:
        _PROGRAM = _build_program()

    if TRACE:
        _install_ntff_hook()
    res = run_bass_kernel_spmd(
        _PROGRAM, in_maps, list(range(NCORES)), trace=TRACE
    )
    LAST_EXEC_NS = res.exec_time_ns
    globals()["LAST_RESULTS"] = res

    # ---- gather / unshard ----
    out_spikes = np.empty((B, NVOTE), dtype=np.float32)
    de_spk = np.zeros((B, T, H), dtype=np.float32)
    for c in range(NCORES):
        r = res.results[c]
        out_spikes[c * BS : (c + 1) * BS] = r["votes"].T
        sp = np.asarray(r["despk"])                         # [NSTEPS,128,NH,BS] bf16
        de_spk[c * BS : (c + 1) * BS, :NSTEPS] = (
            sp.transpose(3, 0, 2, 1).reshape(BS, NSTEPS, H).astype(np.float32)
        )
    return out_spikes, de_spk
